# revision 5
# baseline (speedup 1.0000x reference)
"""Trainium2 Bass kernel for nn_ConvUnit (cimu bit-sliced int8 conv2d).

Reference computation:
  xq = int8(trunc(clip(x, -128, 127)))                    # [32,128,56,56]
  for i in 0..7:
    bit_i = (xq >> i) & 1                                  # {0,1}
    c_i   = conv2d_valid(bit_i, W)                         # [32,128,54,54]
    q_i   = clip(round_half_even(c_i / 2), -128, 127) * 2
    y    += q_i * (2^i  if i < 7 else -128)
  y += bias

Strategy (8 NeuronCores, data-parallel over batch, 4 images/core):
  * ONE fp32r matmul pass per bit plane (HW-probed: fp32r x fp32r
    matmul runs ~1.25 cyc/col at N>=256, only ~7% slower than bf16,
    with ~12-13 effective mantissa bits -- 18x more accurate than
    bf16).  That replaces the baseline's bf16 hi+lo split (18 passes
    -> 8 passes).  Misround probability at 2^-12.6 weight precision
    is ~5e-5, giving rel-err ~1e-3, far under the 2e-2 gate.
  * Weights host-prepped: W/2 scaled by per-plane k_i (power-of-2,
    exact), transposed to lhsT layout [ci, co], kept in f32 and
    bitcast to float32r for the matmul.
  * Conv as 9 shifted matmuls (taps) accumulating in PSUM over
    flattened pixel windows; garbage columns (w>=54) discarded on
    output DMA.
  * round_half_even via the magic-constant trick: since clip never
    fires (checked on host: max_co sum|W|/2 << 127.5),
        u_i = RNE(z + M_i) - M_i  ==  k_i * round_half_even(c_i/2)
    with M_i = 1.5*2^23*|k_i|.  ACT does t = z + M_i (exact f32 add),
    DVE scalar_tensor_tensor fuses (t - M_i) + y.
  * Bit planes in f32 {0,1} (exact): plane 7 is just (x <= -1) on
    DVE; planes 0-6 via exact trunc ladder -> int32 xq -> shift&and.
"""
import sys

sys.path.insert(0, "/opt/trn_rl_repo")

import numpy as np

import concourse.bass as bass
import concourse.tile as tile
from concourse import bacc, mybir
from concourse import bass_utils

N_CORES = 8
B, C, H, W = 32, 128, 56, 56
HO, WO = 54, 54
BPC = B // N_CORES            # images per core
NPIX_IN = H * W               # 3136
NPIX = (HO - 1) * W + WO      # 3022 computed output positions / image
TILE_N = 504                  # 9 output rows x 56 -> row-aligned tiles
ROWS_PER_TILE = 9
TILES = [(j * TILE_N, min(TILE_N, NPIX - j * TILE_N))
         for j in range((NPIX + TILE_N - 1) // TILE_N)]   # 5x504 + 502
# plane 7 first: its bit plane is just (x <= -1), no trunc ladder needed,
# so matmuls start early; the ladder hides behind plane-7 matmuls
PORDER = [7, 0, 1, 2, 3, 4, 5, 6]

MAGIC = 12582912.0            # 1.5 * 2^23: RNE(z + MAGIC) - MAGIC == rhe(z)
# per-plane scale k_i applied to q (folded into weights as k_i/2)
KSCALE = [float(2 << i) for i in range(7)] + [-256.0]

AluOp = mybir.AluOpType
ActFn = mybir.ActivationFunctionType
F32 = mybir.dt.float32
F32R = mybir.dt.float32r
I32 = mybir.dt.int32


# planes computed with 2 fp32r passes (exact rne12 hi + residual lo):
# the PE rounds each operand to 12 mantissa bits (RNE, probed on HW), so
# hi = rne12(w) passes through unchanged and lo = w - hi (exact in f32)
# restores full f32 weight precision across the two accumulated passes.
HILO_PLANES = (6, 7)
NBLK = sum(2 if p in HILO_PLANES else 1 for p in range(8)) * 9


def _rne12(a: np.ndarray) -> np.ndarray:
    man, ex = np.frexp(a.astype(np.float64))
    return np.ldexp(np.round(man * 4096.0) / 4096.0, ex).astype(np.float32)


def _prep_weights(weight: np.ndarray) -> np.ndarray:
    """-> [128ci, NBLK*128co] f32 lhsT blocks, pre-scaled."""
    w2 = weight.astype(np.float32) * np.float32(0.5)
    blocks = []
    for slot, p in enumerate(PORDER):
        s = w2 * np.float32(KSCALE[p])
        if p in HILO_PLANES:
            hi = _rne12(s)
            lo = (s - hi).astype(np.float32)
            halves = (hi, lo)
        else:
            halves = (s,)
        # [co, ci, kh, kw] -> [ci, tap*half, co], tap-major then half
        for tap in range(9):
            for h in halves:
                blocks.append(h[:, :, tap // 3, tap % 3].transpose(1, 0))
    out = np.stack(blocks, axis=1)  # [ci, NBLK, co]
    return np.ascontiguousarray(out.reshape(C, NBLK * C))


def _build(need_clip: bool):
    nc = bacc.Bacc("TRN2", target_bir_lowering=False, debug=False,
                   num_devices=N_CORES)
    xs = nc.dram_tensor("xs", [BPC, C, NPIX_IN], F32, kind="ExternalInput").ap()
    wt = nc.dram_tensor("wt", [C, NBLK * C], F32R, kind="ExternalInput").ap()
    bs = nc.dram_tensor("bs", [C, 1], F32, kind="ExternalInput").ap()
    out = nc.dram_tensor("out", [BPC, C, HO, WO], F32, kind="ExternalOutput").ap()

    with tile.TileContext(nc) as tc:
        with (
            tc.tile_pool(name="wpool", bufs=1) as wpool,
            tc.tile_pool(name="cpool", bufs=1) as cpool,
            tc.tile_pool(name="xpool", bufs=2) as xpool,
            tc.tile_pool(name="tpool", bufs=1) as tpool,
            tc.tile_pool(name="xqpool", bufs=2) as xqpool,
            tc.tile_pool(name="b32pool", bufs=1) as b32pool,
            tc.tile_pool(name="bitpool", bufs=3) as bitpool,
            tc.tile_pool(name="ypool", bufs=2) as ypool,
            tc.tile_pool(name="upool", bufs=6) as upool,
            tc.tile_pool(name="psum", bufs=8, space="PSUM") as pspool,
        ):
            wsb = wpool.tile([C, NBLK * C], F32R)
            # first processed plane's weights land first -> matmuls start early
            nc.sync.dma_start(wsb[:, :18 * C], wt[:, :18 * C])
            nc.sync.dma_start(wsb[:, 18 * C:], wt[:, 18 * C:])
            bsb = cpool.tile([C, 1], F32)
            nc.sync.dma_start(bsb[:], bs[:])

            for img in range(BPC):
                xt = xpool.tile([C, NPIX_IN], F32, tag="x")
                nc.sync.dma_start(xt[:], xs[img])

                # ---- plane 7 bits straight from x: b7 = (x <= -1) ----
                bit7 = bitpool.tile([C, NPIX_IN], F32R, tag="bit")
                nc.vector.tensor_scalar(bit7[:], xt[:], -1.0, None, AluOp.is_le)

                # ---- exact trunc-toward-zero: xq = trunc(clip(x)) ----
                # (hides behind plane-7 matmuls)
                # c = min(max(x, -128), 127)   (in place in xt)
                nc.vector.tensor_scalar(xt[:], xt[:], -128.0, 127.0,
                                        AluOp.max, AluOp.min)
                at = tpool.tile([C, NPIX_IN], F32, tag="ta")   # |c|
                nc.scalar.activation(at[:], xt[:], ActFn.Abs)
                st = tpool.tile([C, NPIX_IN], F32, tag="ts")   # sign(c)
                nc.scalar.activation(st[:], xt[:], ActFn.Sign)
                # f = rhe(|c|)   (reuse xt)
                nc.vector.tensor_scalar(xt[:], at[:], MAGIC, MAGIC,
                                        AluOp.add, AluOp.subtract)
                # g = (f > |c|)  (into at; at dead after)
                nc.vector.tensor_tensor(at[:], xt[:], at[:], AluOp.is_gt)
                # floor(|c|) = f - g   (into xt)
                nc.vector.tensor_tensor(xt[:], xt[:], at[:], AluOp.subtract)
                # trunc(c) = floor(|c|) * sign(c)  (into xt)
                nc.vector.tensor_tensor(xt[:], xt[:], st[:], AluOp.mult)
                # int32 convert (exact: integer-valued input)
                xq = xqpool.tile([C, NPIX_IN], I32, tag="xq")
                nc.vector.tensor_copy(xq[:], xt[:])

                yt = ypool.tile([C, HO * W], F32, tag="y")  # 3024, use 3022

                blk0 = 0
                for slot, plane in enumerate(PORDER):
                    if slot > 0:
                        prev = PORDER[slot - 1]
                        blk0 += 9 * (2 if prev in HILO_PLANES else 1)
                    if plane == 7:
                        bit = bit7
                    else:
                        # ---- bit plane: ((xq >> plane) & 1) as f32r ----
                        b32 = b32pool.tile([C, NPIX_IN], I32, tag="b32")
                        nc.vector.tensor_scalar(b32[:], xq[:], plane, 1,
                                                AluOp.logical_shift_right,
                                                AluOp.bitwise_and)
                        bit = bitpool.tile([C, NPIX_IN], F32R, tag="bit")
                        nc.scalar.copy(bit[:], b32[:])

                    nh = 2 if plane in HILO_PLANES else 1
                    mag = MAGIC * abs(KSCALE[plane])
                    for j, (p0, nj) in enumerate(TILES):
                        ps = pspool.tile([C, TILE_N], F32, tag="ps")
                        for tap in range(9):
                            off = (tap // 3) * W + (tap % 3)
                            for h in range(nh):
                                widx = blk0 + tap * nh + h
                                nc.tensor.matmul(
                                    ps[:, :nj],
                                    wsb[:, widx * C:(widx + 1) * C],
                                    bit[:, p0 + off: p0 + off + nj],
                                    start=(tap == 0 and h == 0),
                                    stop=(tap == 8 and h == nh - 1),
                                )
                        yv = yt[:, p0:p0 + nj]
                        if slot == 0:
                            # y = rhe(psum) * k  directly from PSUM on DVE
                            nc.vector.tensor_scalar(yv, ps[:, :nj], mag, mag,
                                                    AluOp.add, AluOp.subtract)
                        else:
                            # ACT: t = psum + M   (RNE -> rounds to mult of k)
                            ut = upool.tile([C, TILE_N], F32, tag="u")
                            nc.scalar.activation(ut[:, :nj], ps[:, :nj],
                                                 ActFn.Copy, bias=mag)
                            if need_clip:
                                lok, hik = ((-128.0, 127.0)
                                            if KSCALE[plane] > 0 else (-127.0, 128.0))
                                nc.vector.tensor_scalar(
                                    ut[:, :nj], ut[:, :nj],
                                    mag + lok * abs(KSCALE[plane]),
                                    mag + hik * abs(KSCALE[plane]),
                                    AluOp.max, AluOp.min)
                            # y = (t - M) + y   fused on DVE
                            nc.vector.scalar_tensor_tensor(
                                yv, ut[:, :nj], mag, yv,
                                AluOp.subtract, AluOp.add)
                        if slot == 7:
                            # last plane: bias + per-tile writeout (tiles are
                            # row-aligned: 9 output rows each)
                            nc.vector.tensor_scalar(yv, yv, bsb[:, 0:1], None,
                                                    AluOp.add)
                            r0 = j * ROWS_PER_TILE
                            ysrc = yt[:].rearrange("p (h w) -> p h w", w=W)[
                                :, r0:r0 + ROWS_PER_TILE, 0:WO]
                            nc.sync.dma_start(out[img][:, r0:r0 + ROWS_PER_TILE, :],
                                              ysrc)

    nc.compile()
    return nc


_CACHE = {}


def _get_nc(need_clip: bool):
    if need_clip not in _CACHE:
        _CACHE[need_clip] = _build(need_clip)
    return _CACHE[need_clip]


def kernel(x: np.ndarray, weight: np.ndarray, bias: np.ndarray,
           _trace: bool = False):
    x = np.ascontiguousarray(x, dtype=np.float32)
    weight = np.ascontiguousarray(weight, dtype=np.float32)
    bias = np.ascontiguousarray(bias, dtype=np.float32)

    w_host = _prep_weights(weight)
    # clip in the reference only fires if |conv/2| can reach 127.5
    need_clip = float(np.abs(weight).sum(axis=(1, 2, 3)).max()) * 0.5 >= 127.4
    nc = _get_nc(need_clip)

    bs_host = bias.reshape(C, 1)
    xr = x.reshape(B, C, NPIX_IN)
    in_maps = []
    for c in range(N_CORES):
        in_maps.append({
            "xs": np.ascontiguousarray(xr[c * BPC:(c + 1) * BPC]),
            "wt": w_host,
            "bs": bs_host,
        })

    res = bass_utils.run_bass_kernel_spmd(
        nc, in_maps, core_ids=list(range(N_CORES)), trace=_trace)

    y = np.concatenate([res.results[c]["out"] for c in range(N_CORES)], axis=0)
    if _trace:
        kernel._last_results = res
    return y


if __name__ == "__main__":
    np.random.seed(0)
    x = (np.random.randn(B, C, H, W) * 60).astype(np.float32)
    w = (np.random.randn(C, C, 3, 3) * 0.05).astype(np.float32)
    b = np.random.randn(C).astype(np.float32)
    y = kernel(x, w, b)
    print("out", y.shape, y.dtype)
